# revision 27
# baseline (speedup 1.0000x reference)
"""LowRankAttention Trainium2 kernel (Bass/Tile), data-parallel over 8 NeuronCores.

Math per batch b (one batch per core):
    Q = q @ Wq^T, K = k @ Wk^T, V = v @ Wv^T          (rank projections, R=256)
    A = softmax(Q K^T / sqrt(R))                       (softmax over keys j)
    out = (A @ V) @ Wo^T

All-bf16 dataflow (PE runs bf16 at 1 cycle/row; ~half the f32 DMA traffic;
output written bf16 and upcast on host).  Q^T/K^T kept as [R, S], V as
[S, R]; A^T tiles [j, i] from lhsT=K^T, rhs=Q^T; exp() on ScalarE -> bf16
et tiles (no max-subtraction: |logits| < ~7); EV accumulates AV^T in PSUM.
Row sums via DVE accumulation of et tiles + 4 single-column f32 matmuls per
chunk landing sums directly in the [i_p, 1] layout the per-partition output
scale needs; 1/sum folded into the output-projection copy.

v2 changes vs the 165us baseline:
 - Input DMAs split per s-chunk (1 MiB each) and interleaved in PE
   consumption order, so the first K-projection matmul waits on ~1.5 MiB,
   not the full 4 MiB k load (was a 16 us PE stall at startup).
 - All non-AV PSUM tiles share one 6-slot ring (single tag) + 2 AV banks =
   8 banks, giving the output-projection stage a deep enough rotation that
   PE no longer stalls ~650 ns per PSUM reuse against the scale copies.
 - Last chunk's output projection runs it-major with per-it avt copies,
   alternating scale engines, and 4 small output DMAs on the idle SP/HWDGE
   queue to shorten the post-PE tail.

Engine split respects GPSIMD's no-PSUM rule: ScalarE = exp + PSUM->SBUF
copies + half the scales; DVE = acc chain/avt/reciprocal/other scales;
GPSIMD = steady-state output DMA (SWDGE); SP = input DMAs + tail output.
"""

import numpy as np
import ml_dtypes

import concourse.bacc as bacc
import concourse.mybir as mybir
import concourse.tile as tile
from concourse import bass_utils

F32 = mybir.dt.float32
BF16 = mybir.dt.bfloat16
AF = mybir.ActivationFunctionType
ADD = mybir.AluOpType.add

DIM, RANK, B, S = 1024, 256, 8, 2048
P = 128
NC = 512                      # moving-operand / psum free chunk
DT = DIM // P                 # 8  d-tiles
RT = RANK // P                # 2  r-tiles
SC = S // NC                  # 4  s-chunks (i-chunks)
JT = S // P                   # 16 j-tiles
JC = NC // P                  # 4  j-tiles per s-chunk
SCALE = 1.0 / np.sqrt(np.float32(RANK))
LAG = 4
NWARM = 10                    # PE p-state warmup matmuls on zeroed SBUF


def build_program(reps: int = 1):
    """Build + compile the per-core Bass program. reps>1 wraps the whole body
    in a For_i loop (used only for wall-clock timing)."""
    nc = bacc.Bacc("TRN2", target_bir_lowering=False, debug=False)

    qT = nc.dram_tensor("qT", [DIM, S], BF16, kind="ExternalInput")
    kT = nc.dram_tensor("kT", [DIM, S], BF16, kind="ExternalInput")
    vT = nc.dram_tensor("vT", [DIM, S], BF16, kind="ExternalInput")
    wqT = nc.dram_tensor("wqT", [DIM, RANK], BF16, kind="ExternalInput")
    wkT = nc.dram_tensor("wkT", [DIM, RANK], BF16, kind="ExternalInput")
    wvT = nc.dram_tensor("wvT", [DIM, RANK], BF16, kind="ExternalInput")
    woT = nc.dram_tensor("woT", [RANK, DIM], BF16, kind="ExternalInput")
    out = nc.dram_tensor("out", [S, DIM], BF16, kind="ExternalOutput")

    with tile.TileContext(nc) as tc:
        with tc.tile_pool(name="w", bufs=1) as wpool, \
             tc.tile_pool(name="kin", bufs=1) as kpool, \
             tc.tile_pool(name="vin", bufs=1) as vpool, \
             tc.tile_pool(name="qin", bufs=1) as qpool, \
             tc.tile_pool(name="per", bufs=1) as perpool, \
             tc.tile_pool(name="et", bufs=6) as etpool, \
             tc.tile_pool(name="acc", bufs=2) as accpool, \
             tc.tile_pool(name="av", bufs=4) as avpool, \
             tc.tile_pool(name="inv", bufs=2) as invpool, \
             tc.tile_pool(name="o", bufs=2) as opool, \
             tc.tile_pool(name="ps", bufs=6, space="PSUM") as pspool, \
             tc.tile_pool(name="psav", bufs=2, space="PSUM") as psavpool:

            def ps_tile(name):
                return pspool.tile([P, NC], F32, tag="ps", name=name)

            def body(_i=None, warm=True):
                # ---- weights + constants ----
                wq_t = wpool.tile([P, DT, RANK], BF16, tag="wq", name="wq_t")
                wk_t = wpool.tile([P, DT, RANK], BF16, tag="wk", name="wk_t")
                wv_t = wpool.tile([P, DT, RANK], BF16, tag="wv", name="wv_t")
                wo_t = wpool.tile([P, RT, DIM], BF16, tag="wo", name="wo_t")
                ones_f = wpool.tile([P, 1], F32, tag="onesf", name="ones_f")
                nc.vector.memset(ones_f[:], 1.0)

                # PE p-state warmup: matmuls over a zeroed tile keep the PE
                # array busy (and its frequency ramping) while the first input
                # DMAs land.  Results go to rotating PSUM slots, never read.
                if warm:
                    zt = wpool.tile([P, NC], BF16, tag="zt", name="zt")
                    nc.vector.memset(zt[:], 0.0)
                    for _w in range(NWARM):
                        pw = ps_tile("ps_warm")
                        nc.tensor.matmul(pw[:], zt[:, :P], zt[:], start=True,
                                         stop=True)

                # ---- chunked input DMAs, interleaved in PE-consumption order
                kch = [kpool.tile([P, DT, NC], BF16, tag=f"kin{c}", name=f"kin{c}")
                       for c in range(SC)]
                vch = [vpool.tile([P, DT, NC], BF16, tag=f"vin{c}", name=f"vin{c}")
                       for c in range(SC)]
                qch = [qpool.tile([P, DT, NC], BF16, tag=f"qin{c}", name=f"qin{c}")
                       for c in range(SC)]
                kT_r = kT.ap().rearrange("(dt p) s -> p dt s", p=P)
                vT_r = vT.ap().rearrange("(dt p) s -> p dt s", p=P)
                qT_r = qT.ap().rearrange("(dt p) s -> p dt s", p=P)

                def load_chunk(dst, src_r, c, splits=1):
                    w = NC // splits
                    for h in range(splits):
                        sl = slice(h * w, (h + 1) * w)
                        nc.sync.dma_start(dst[:, :, sl],
                                          src_r[:, :, c * NC + h * w:c * NC + (h + 1) * w])

                # Single SP queue in exact PE-consumption order (a second DGE
                # queue would let weight transfers cut ahead of the k stream);
                # k0 split in half so the first K-projection pass waits ~1 MiB.
                nc.sync.dma_start(wk_t[:], wkT.ap().rearrange("(dt p) r -> p dt r", p=P))
                load_chunk(kch[0], kT_r, 0, splits=2 if warm else 1)
                load_chunk(kch[1], kT_r, 1)
                load_chunk(kch[2], kT_r, 2)
                load_chunk(kch[3], kT_r, 3)
                nc.sync.dma_start(wv_t[:], wvT.ap().rearrange("(dt p) r -> p dt r", p=P))
                load_chunk(vch[0], vT_r, 0)
                load_chunk(vch[1], vT_r, 1)
                load_chunk(vch[2], vT_r, 2)
                load_chunk(vch[3], vT_r, 3)
                nc.sync.dma_start(wq_t[:], wqT.ap().rearrange("(dt p) r -> p dt r", p=P))
                load_chunk(qch[0], qT_r, 0)
                nc.sync.dma_start(wo_t[:], woT.ap().rearrange("(rt p) d -> p rt d", p=P))
                load_chunk(qch[1], qT_r, 1)
                load_chunk(qch[2], qT_r, 2)
                load_chunk(qch[3], qT_r, 3)

                # ---- persistent projections ----
                QT_t = perpool.tile([P, RT, S], BF16, tag="QT", name="QT_t")   # [r_p, rt, i]
                KT_t = perpool.tile([P, RT, S], BF16, tag="KT", name="KT_t")   # [r_p, rt, j]
                V_t = perpool.tile([P, JT, RANK], BF16, tag="V", name="V_t")   # [j_p, jt, r]

                # K^T projection, per s-chunk (chunk 0 in two half-width
                # passes so it can start on the first half-DMA)
                for sc in range(SC):
                    pss = [ps_tile("ps_projk") for _ in range(RT)]
                    halves = 2 if (sc == 0 and warm) else 1
                    w = NC // halves
                    for h in range(halves):
                        sl = slice(h * w, (h + 1) * w)
                        for dt in range(DT):
                            for rt in range(RT):
                                nc.tensor.matmul(pss[rt][:, sl],
                                                 wk_t[:, dt, rt * P:(rt + 1) * P],
                                                 kch[sc][:, dt, sl],
                                                 start=(dt == 0), stop=(dt == DT - 1))
                    for rt in range(RT):
                        if rt == 0:
                            nc.scalar.copy(KT_t[:, rt, sc * NC:(sc + 1) * NC], pss[rt][:])
                        else:
                            nc.vector.tensor_copy(KT_t[:, rt, sc * NC:(sc + 1) * NC],
                                                  pss[rt][:])

                # V projection, per j-tile pair
                for jt0 in range(0, JT, 2):
                    psvs = []
                    for j in (jt0, jt0 + 1):
                        ps = ps_tile("ps_v")
                        psvs.append(ps[:, :RANK])
                    for dt in range(DT):
                        for i, j in enumerate((jt0, jt0 + 1)):
                            nc.tensor.matmul(
                                psvs[i],
                                vch[j // JC][:, dt, (j % JC) * P:(j % JC + 1) * P],
                                wv_t[:, dt, :],
                                start=(dt == 0), stop=(dt == DT - 1))
                    for i, j in enumerate((jt0, jt0 + 1)):
                        if i == 0:
                            nc.scalar.copy(V_t[:, j, :], psvs[i])
                        else:
                            nc.vector.tensor_copy(V_t[:, j, :], psvs[i])

                # ---- per-chunk: Q^T proj + attention (lag-N A^T -> EV pipeline).
                # outproj of chunk ic-1 is emitted after Qproj(ic) so the PE has
                # work while the QT copy lands.
                pending = {}

                def emit_qproj(ic):
                    isl = slice(ic * NC, (ic + 1) * NC)
                    pss = [ps_tile("ps_projq") for _ in range(RT)]
                    for dt in range(DT):
                        for rt in range(RT):
                            nc.tensor.matmul(pss[rt][:], wq_t[:, dt, rt * P:(rt + 1) * P],
                                             qch[ic][:, dt, :],
                                             start=(dt == 0), stop=(dt == DT - 1))
                    for rt in range(RT):
                        if rt == 0:
                            nc.scalar.copy(QT_t[:, rt, isl], pss[rt][:])
                        else:
                            nc.vector.tensor_copy(QT_t[:, rt, isl], pss[rt][:])

                def emit_rowsums(ic):
                    av_ps, acc = pending.pop(ic)
                    # row sums -> [i_p, 1] per i-tile (PSUM), then reciprocal.
                    rs = ps_tile("rs")
                    for it in range(JC):
                        nc.tensor.matmul(rs[:, it:it + 1], acc[:, it * P:(it + 1) * P],
                                         ones_f[:], start=True, stop=True)
                    inv = invpool.tile([P, SC], F32, tag="inv", name="inv")
                    nc.vector.reciprocal(inv[:], rs[:, :JC])
                    return av_ps, inv

                def emit_outproj(ic):
                    av_ps, inv = emit_rowsums(ic)
                    avt_sb = []
                    for rt in range(RT):
                        t = avpool.tile([P, NC], BF16, tag="avt", name=f"avt_{rt}")
                        nc.vector.tensor_copy(t[:], av_ps[rt][:])
                        avt_sb.append(t)
                    out_r = out.ap().rearrange("(it p) d -> p it d", p=P)
                    for itp in range(0, JC, 2):
                        its = (itp, itp + 1)
                        otp = opool.tile([P, 2, DIM], BF16, tag="out", name="ot")
                        for dc in range(DIM // NC):
                            pss = {it: ps_tile("ps_o") for it in its}
                            for rt in range(RT):
                                for it in its:
                                    nc.tensor.matmul(
                                        pss[it][:], avt_sb[rt][:, it * P:(it + 1) * P],
                                        wo_t[:, rt, dc * NC:(dc + 1) * NC],
                                        start=(rt == 0), stop=(rt == RT - 1))
                            for i, it in enumerate(its):
                                if i == 0:
                                    nc.vector.tensor_scalar_mul(
                                        otp[:, i, dc * NC:(dc + 1) * NC],
                                        pss[it][:], inv[:, it:it + 1])
                                else:
                                    nc.scalar.mul(otp[:, i, dc * NC:(dc + 1) * NC],
                                                  pss[it][:], inv[:, it:it + 1])
                        gi = ic * JC + itp
                        nc.gpsimd.dma_start(out_r[:, gi:gi + 2, :], otp[:])

                def emit_outproj_last(ic):
                    # it-major with per-it avt copies and small sync-queue DMAs:
                    # shortens the critical path from the last EV matmul to the
                    # final output byte.
                    av_ps, inv = emit_rowsums(ic)
                    out_r = out.ap().rearrange("(it p) d -> p it d", p=P)
                    avt_sb = [avpool.tile([P, NC], BF16, tag="avt", name=f"avt_{rt}")
                              for rt in range(RT)]
                    for it in range(JC):
                        isl = slice(it * P, (it + 1) * P)
                        for rt in range(RT):
                            if it % 2 == 0:
                                nc.vector.tensor_copy(avt_sb[rt][:, isl], av_ps[rt][:, isl])
                            else:
                                nc.scalar.copy(avt_sb[rt][:, isl], av_ps[rt][:, isl])
                        otp = opool.tile([P, 1, DIM], BF16, tag="outl", bufs=4,
                                         name="otl")
                        for dc in range(DIM // NC):
                            ps = ps_tile("ps_o")
                            for rt in range(RT):
                                nc.tensor.matmul(
                                    ps[:], avt_sb[rt][:, isl],
                                    wo_t[:, rt, dc * NC:(dc + 1) * NC],
                                    start=(rt == 0), stop=(rt == RT - 1))
                            if (it + dc) % 2 == 0:
                                nc.vector.tensor_scalar_mul(
                                    otp[:, 0, dc * NC:(dc + 1) * NC], ps[:],
                                    inv[:, it:it + 1])
                            else:
                                nc.scalar.mul(otp[:, 0, dc * NC:(dc + 1) * NC], ps[:],
                                              inv[:, it:it + 1])
                        gi = ic * JC + it
                        q = nc.sync if warm else nc.gpsimd
                        q.dma_start(out_r[:, gi:gi + 1, :], otp[:])

                def emit_attention(ic):
                    isl = slice(ic * NC, (ic + 1) * NC)
                    av_ps = [psavpool.tile([P, NC], F32, tag="av", name=f"av_{rt}")
                             for rt in range(RT)]
                    acc = accpool.tile([P, NC], F32, tag="acc", name="acc")
                    ets = {}

                    def step(jt):
                        do_at, do_ev = jt < JT, jt >= LAG
                        ej = jt - LAG
                        if do_at:
                            ps = ps_tile("ps_at")
                        if do_ev:
                            et = ets.pop(ej)
                        if do_at:
                            nc.tensor.matmul(ps[:], KT_t[:, 0, jt * P:(jt + 1) * P],
                                             QT_t[:, 0, isl], start=True, stop=False)
                        if do_ev:
                            nc.tensor.matmul(av_ps[0][:], V_t[:, ej, 0:P], et[:],
                                             start=(ej == 0), stop=(ej == JT - 1))
                        if do_at:
                            nc.tensor.matmul(ps[:], KT_t[:, 1, jt * P:(jt + 1) * P],
                                             QT_t[:, 1, isl], start=False, stop=True)
                        if do_ev:
                            nc.tensor.matmul(av_ps[1][:], V_t[:, ej, P:RANK], et[:],
                                             start=(ej == 0), stop=(ej == JT - 1))
                        if do_at:
                            etn = etpool.tile([P, NC], BF16, tag="et", name="et")
                            nc.scalar.activation(etn[:], ps[:], AF.Exp, scale=float(SCALE))
                            ets[jt] = etn
                        if do_ev:
                            if ej == 0:
                                nc.vector.tensor_copy(acc[:], et[:])
                            else:
                                nc.vector.tensor_tensor(acc[:], acc[:], et[:], ADD)

                    for jt in range(JT + LAG):
                        step(jt)
                    pending[ic] = (av_ps, acc)

                for ic in range(SC):
                    emit_qproj(ic)
                    if ic > 0:
                        emit_outproj(ic - 1)
                    emit_attention(ic)
                emit_outproj_last(SC - 1)

            if reps == 1:
                body()
            else:
                # Steady-state loop: PE stays warm across iterations, so the
                # p-state warmup matmuls would be pure per-iteration overhead.
                with tc.For_i(0, reps, 1) as i:
                    body(i, warm=False)

    nc.compile()
    return nc


_CACHE = {}


def _get_program():
    if "nc" not in _CACHE:
        _CACHE["nc"] = build_program(reps=1)
    return _CACHE["nc"]


def _bf16(x):
    return np.asarray(x, dtype=np.float32).astype(ml_dtypes.bfloat16)


def prep_in_maps(q, k, v, Wq, Wk, Wv, Wo):
    # Zero-FLOP host-side layout/dtype prep: transpose so the contraction dim
    # (D) lands on SBUF partitions, cast to bf16; one batch per core.
    qT = np.asarray(q, dtype=np.float32).transpose(0, 2, 1).astype(ml_dtypes.bfloat16)
    kT = np.asarray(k, dtype=np.float32).transpose(0, 2, 1).astype(ml_dtypes.bfloat16)
    vT = np.asarray(v, dtype=np.float32).transpose(0, 2, 1).astype(ml_dtypes.bfloat16)
    wqT = _bf16(Wq).T.copy()
    wkT = _bf16(Wk).T.copy()
    wvT = _bf16(Wv).T.copy()
    woT = _bf16(Wo).T.copy()
    return [{"qT": qT[c], "kT": kT[c], "vT": vT[c],
             "wqT": wqT, "wkT": wkT, "wvT": wvT, "woT": woT}
            for c in range(B)]


def kernel(q, k, v, Wq, Wk, Wv, Wo):
    nc = _get_program()
    in_maps = prep_in_maps(q, k, v, Wq, Wk, Wv, Wo)
    res = bass_utils.run_bass_kernel_spmd(nc, in_maps, core_ids=list(range(B)))
    return np.stack([res.results[c]["out"] for c in range(B)], axis=0).astype(np.float32)


# revision 28
# speedup vs baseline: 1.0239x; 1.0239x over previous
"""LowRankAttention Trainium2 kernel (Bass/Tile), data-parallel over 8 NeuronCores.

Math per batch b (one batch per core):
    Q = q @ Wq^T, K = k @ Wk^T, V = v @ Wv^T          (rank projections, R=256)
    A = softmax(Q K^T / sqrt(R))                       (softmax over keys j)
    out = (A @ V) @ Wo^T

All-bf16 dataflow (PE runs bf16 at 1 cycle/row; ~half the f32 DMA traffic;
output written bf16 and upcast on host).  Q^T/K^T kept as [R, S], V as
[S, R]; A^T tiles [j, i] from lhsT=K^T, rhs=Q^T; exp() on ScalarE -> bf16
et tiles (no max-subtraction: |logits| < ~7); EV accumulates AV^T in PSUM.
Row sums via DVE accumulation of et tiles + 4 single-column f32 matmuls per
chunk landing sums directly in the [i_p, 1] layout the per-partition output
scale needs; 1/sum folded into the output-projection copy.

Changes vs the 165us baseline:
 - Input DMAs split per s-chunk (1 MiB each; k0 halved) and issued on the
   single SP queue in exact PE consumption order, so the first K-projection
   matmul waits on ~1.5 MiB, not the full 4 MiB k load (was a 16 us PE
   stall at startup).
 - All non-AV PSUM tiles share one 6-slot ring (single tag) + 2 AV banks =
   8 banks, giving the output-projection stage a deep enough rotation that
   PE no longer stalls ~650 ns per PSUM reuse against the scale copies.
 - 8 warmup matmuls on a zeroed tile ramp the PE p-state while the first
   DMAs land (and, in the For_i timing loop, while the previous iteration's
   tail drains) -- A/B measured ~3 us/iter faster with them.
 - PSUM->SBUF copies alternate ScalarE/DVE so neither engine's queue backs
   up at chunk boundaries.
 - Last chunk's output projection runs it-major with per-it avt copies,
   alternating scale engines, and 4 small output DMAs on the idle SP/HWDGE
   queue to shorten the post-PE tail.
 - Matmul count matters: ~42 ns/instr fixed overhead measured on HW, so
   free dims are 512 (PSUM-bank max) everywhere the pipeline allows.

Engine split respects GPSIMD's no-PSUM rule: ScalarE = exp + PSUM->SBUF
copies + half the scales; DVE = acc chain/avt/reciprocal/other scales;
GPSIMD = steady-state output DMA (SWDGE); SP = input DMAs + tail output.
"""

import numpy as np
import ml_dtypes

import concourse.bacc as bacc
import concourse.mybir as mybir
import concourse.tile as tile
from concourse import bass_utils

F32 = mybir.dt.float32
BF16 = mybir.dt.bfloat16
AF = mybir.ActivationFunctionType
ADD = mybir.AluOpType.add

DIM, RANK, B, S = 1024, 256, 8, 2048
P = 128
NC = 512                      # moving-operand / psum free chunk
DT = DIM // P                 # 8  d-tiles
RT = RANK // P                # 2  r-tiles
SC = S // NC                  # 4  s-chunks (i-chunks)
JT = S // P                   # 16 j-tiles
JC = NC // P                  # 4  j-tiles per s-chunk
SCALE = 1.0 / np.sqrt(np.float32(RANK))
LAG = 4
NWARM = 8                     # PE p-state warmup matmuls on zeroed SBUF


def build_program(reps: int = 1):
    """Build + compile the per-core Bass program. reps>1 wraps the whole body
    in a For_i loop (used only for wall-clock timing)."""
    nc = bacc.Bacc("TRN2", target_bir_lowering=False, debug=False)

    qT = nc.dram_tensor("qT", [DIM, S], BF16, kind="ExternalInput")
    kT = nc.dram_tensor("kT", [DIM, S], BF16, kind="ExternalInput")
    vT = nc.dram_tensor("vT", [DIM, S], BF16, kind="ExternalInput")
    wqT = nc.dram_tensor("wqT", [DIM, RANK], BF16, kind="ExternalInput")
    wkT = nc.dram_tensor("wkT", [DIM, RANK], BF16, kind="ExternalInput")
    wvT = nc.dram_tensor("wvT", [DIM, RANK], BF16, kind="ExternalInput")
    woT = nc.dram_tensor("woT", [RANK, DIM], BF16, kind="ExternalInput")
    out = nc.dram_tensor("out", [S, DIM], BF16, kind="ExternalOutput")

    with tile.TileContext(nc) as tc:
        with tc.tile_pool(name="w", bufs=1) as wpool, \
             tc.tile_pool(name="kin", bufs=1) as kpool, \
             tc.tile_pool(name="vin", bufs=1) as vpool, \
             tc.tile_pool(name="qin", bufs=1) as qpool, \
             tc.tile_pool(name="per", bufs=1) as perpool, \
             tc.tile_pool(name="et", bufs=6) as etpool, \
             tc.tile_pool(name="acc", bufs=2) as accpool, \
             tc.tile_pool(name="av", bufs=4) as avpool, \
             tc.tile_pool(name="inv", bufs=2) as invpool, \
             tc.tile_pool(name="o", bufs=2) as opool, \
             tc.tile_pool(name="ps", bufs=6, space="PSUM") as pspool, \
             tc.tile_pool(name="psav", bufs=2, space="PSUM") as psavpool:

            def ps_tile(name):
                return pspool.tile([P, NC], F32, tag="ps", name=name)

            def body(_i=None, warm=True):
                # ---- weights + constants ----
                wq_t = wpool.tile([P, DT, RANK], BF16, tag="wq", name="wq_t")
                wk_t = wpool.tile([P, DT, RANK], BF16, tag="wk", name="wk_t")
                wv_t = wpool.tile([P, DT, RANK], BF16, tag="wv", name="wv_t")
                wo_t = wpool.tile([P, RT, DIM], BF16, tag="wo", name="wo_t")
                ones_f = wpool.tile([P, 1], F32, tag="onesf", name="ones_f")
                nc.vector.memset(ones_f[:], 1.0)

                # PE p-state warmup: matmuls over a zeroed tile keep the PE
                # array busy (and its frequency ramping) while the first input
                # DMAs land.  Results go to rotating PSUM slots, never read.
                if warm:
                    zt = wpool.tile([P, NC], BF16, tag="zt", name="zt")
                    nc.vector.memset(zt[:], 0.0)
                    for _w in range(NWARM):
                        pw = ps_tile("ps_warm")
                        nc.tensor.matmul(pw[:], zt[:, :P], zt[:], start=True,
                                         stop=True)

                # ---- chunked input DMAs, interleaved in PE-consumption order
                kch = [kpool.tile([P, DT, NC], BF16, tag=f"kin{c}", name=f"kin{c}")
                       for c in range(SC)]
                vch = [vpool.tile([P, DT, NC], BF16, tag=f"vin{c}", name=f"vin{c}")
                       for c in range(SC)]
                qch = [qpool.tile([P, DT, NC], BF16, tag=f"qin{c}", name=f"qin{c}")
                       for c in range(SC)]
                kT_r = kT.ap().rearrange("(dt p) s -> p dt s", p=P)
                vT_r = vT.ap().rearrange("(dt p) s -> p dt s", p=P)
                qT_r = qT.ap().rearrange("(dt p) s -> p dt s", p=P)

                def load_chunk(dst, src_r, c, splits=1):
                    w = NC // splits
                    for h in range(splits):
                        sl = slice(h * w, (h + 1) * w)
                        nc.sync.dma_start(dst[:, :, sl],
                                          src_r[:, :, c * NC + h * w:c * NC + (h + 1) * w])

                # Single SP queue in exact PE-consumption order (a second DGE
                # queue would let weight transfers cut ahead of the k stream);
                # k0 split in half so the first K-projection pass waits ~1 MiB.
                nc.sync.dma_start(wk_t[:], wkT.ap().rearrange("(dt p) r -> p dt r", p=P))
                load_chunk(kch[0], kT_r, 0, splits=2 if warm else 1)
                load_chunk(kch[1], kT_r, 1)
                load_chunk(kch[2], kT_r, 2)
                load_chunk(kch[3], kT_r, 3)
                nc.sync.dma_start(wv_t[:], wvT.ap().rearrange("(dt p) r -> p dt r", p=P))
                load_chunk(vch[0], vT_r, 0)
                load_chunk(vch[1], vT_r, 1)
                load_chunk(vch[2], vT_r, 2)
                load_chunk(vch[3], vT_r, 3)
                nc.sync.dma_start(wq_t[:], wqT.ap().rearrange("(dt p) r -> p dt r", p=P))
                load_chunk(qch[0], qT_r, 0)
                nc.sync.dma_start(wo_t[:], woT.ap().rearrange("(rt p) d -> p rt d", p=P))
                load_chunk(qch[1], qT_r, 1)
                load_chunk(qch[2], qT_r, 2)
                load_chunk(qch[3], qT_r, 3)

                # ---- persistent projections ----
                QT_t = perpool.tile([P, RT, S], BF16, tag="QT", name="QT_t")   # [r_p, rt, i]
                KT_t = perpool.tile([P, RT, S], BF16, tag="KT", name="KT_t")   # [r_p, rt, j]
                V_t = perpool.tile([P, JT, RANK], BF16, tag="V", name="V_t")   # [j_p, jt, r]

                # K^T projection, per s-chunk (chunk 0 in two half-width
                # passes so it can start on the first half-DMA)
                for sc in range(SC):
                    pss = [ps_tile("ps_projk") for _ in range(RT)]
                    halves = 2 if (sc == 0 and warm) else 1
                    w = NC // halves
                    for h in range(halves):
                        sl = slice(h * w, (h + 1) * w)
                        for dt in range(DT):
                            for rt in range(RT):
                                nc.tensor.matmul(pss[rt][:, sl],
                                                 wk_t[:, dt, rt * P:(rt + 1) * P],
                                                 kch[sc][:, dt, sl],
                                                 start=(dt == 0), stop=(dt == DT - 1))
                    for rt in range(RT):
                        if rt == 0:
                            nc.scalar.copy(KT_t[:, rt, sc * NC:(sc + 1) * NC], pss[rt][:])
                        else:
                            nc.vector.tensor_copy(KT_t[:, rt, sc * NC:(sc + 1) * NC],
                                                  pss[rt][:])

                # V projection, per j-tile pair
                for jt0 in range(0, JT, 2):
                    psvs = []
                    for j in (jt0, jt0 + 1):
                        ps = ps_tile("ps_v")
                        psvs.append(ps[:, :RANK])
                    for dt in range(DT):
                        for i, j in enumerate((jt0, jt0 + 1)):
                            nc.tensor.matmul(
                                psvs[i],
                                vch[j // JC][:, dt, (j % JC) * P:(j % JC + 1) * P],
                                wv_t[:, dt, :],
                                start=(dt == 0), stop=(dt == DT - 1))
                    for i, j in enumerate((jt0, jt0 + 1)):
                        if i == 0:
                            nc.scalar.copy(V_t[:, j, :], psvs[i])
                        else:
                            nc.vector.tensor_copy(V_t[:, j, :], psvs[i])

                # ---- per-chunk: Q^T proj + attention (lag-N A^T -> EV pipeline).
                # outproj of chunk ic-1 is emitted after Qproj(ic) so the PE has
                # work while the QT copy lands.
                pending = {}

                def emit_qproj(ic):
                    isl = slice(ic * NC, (ic + 1) * NC)
                    pss = [ps_tile("ps_projq") for _ in range(RT)]
                    for dt in range(DT):
                        for rt in range(RT):
                            nc.tensor.matmul(pss[rt][:], wq_t[:, dt, rt * P:(rt + 1) * P],
                                             qch[ic][:, dt, :],
                                             start=(dt == 0), stop=(dt == DT - 1))
                    for rt in range(RT):
                        if rt == 0:
                            nc.scalar.copy(QT_t[:, rt, isl], pss[rt][:])
                        else:
                            nc.vector.tensor_copy(QT_t[:, rt, isl], pss[rt][:])

                def emit_rowsums(ic):
                    av_ps, acc = pending.pop(ic)
                    # row sums -> [i_p, 1] per i-tile (PSUM), then reciprocal.
                    rs = ps_tile("rs")
                    for it in range(JC):
                        nc.tensor.matmul(rs[:, it:it + 1], acc[:, it * P:(it + 1) * P],
                                         ones_f[:], start=True, stop=True)
                    inv = invpool.tile([P, SC], F32, tag="inv", name="inv")
                    nc.vector.reciprocal(inv[:], rs[:, :JC])
                    return av_ps, inv

                def emit_outproj(ic):
                    av_ps, inv = emit_rowsums(ic)
                    avt_sb = []
                    for rt in range(RT):
                        t = avpool.tile([P, NC], BF16, tag="avt", name=f"avt_{rt}")
                        nc.vector.tensor_copy(t[:], av_ps[rt][:])
                        avt_sb.append(t)
                    out_r = out.ap().rearrange("(it p) d -> p it d", p=P)
                    for itp in range(0, JC, 2):
                        its = (itp, itp + 1)
                        otp = opool.tile([P, 2, DIM], BF16, tag="out", name="ot")
                        for dc in range(DIM // NC):
                            pss = {it: ps_tile("ps_o") for it in its}
                            for rt in range(RT):
                                for it in its:
                                    nc.tensor.matmul(
                                        pss[it][:], avt_sb[rt][:, it * P:(it + 1) * P],
                                        wo_t[:, rt, dc * NC:(dc + 1) * NC],
                                        start=(rt == 0), stop=(rt == RT - 1))
                            for i, it in enumerate(its):
                                if i == 0:
                                    nc.vector.tensor_scalar_mul(
                                        otp[:, i, dc * NC:(dc + 1) * NC],
                                        pss[it][:], inv[:, it:it + 1])
                                else:
                                    nc.scalar.mul(otp[:, i, dc * NC:(dc + 1) * NC],
                                                  pss[it][:], inv[:, it:it + 1])
                        gi = ic * JC + itp
                        nc.gpsimd.dma_start(out_r[:, gi:gi + 2, :], otp[:])

                def emit_outproj_last(ic):
                    # it-major with per-it avt copies and small sync-queue DMAs:
                    # shortens the critical path from the last EV matmul to the
                    # final output byte.
                    av_ps, inv = emit_rowsums(ic)
                    out_r = out.ap().rearrange("(it p) d -> p it d", p=P)
                    avt_sb = [avpool.tile([P, NC], BF16, tag="avt", name=f"avt_{rt}")
                              for rt in range(RT)]
                    for it in range(JC):
                        isl = slice(it * P, (it + 1) * P)
                        for rt in range(RT):
                            if it % 2 == 0:
                                nc.vector.tensor_copy(avt_sb[rt][:, isl], av_ps[rt][:, isl])
                            else:
                                nc.scalar.copy(avt_sb[rt][:, isl], av_ps[rt][:, isl])
                        otp = opool.tile([P, 1, DIM], BF16, tag="outl", bufs=4,
                                         name="otl")
                        for dc in range(DIM // NC):
                            ps = ps_tile("ps_o")
                            for rt in range(RT):
                                nc.tensor.matmul(
                                    ps[:], avt_sb[rt][:, isl],
                                    wo_t[:, rt, dc * NC:(dc + 1) * NC],
                                    start=(rt == 0), stop=(rt == RT - 1))
                            if (it + dc) % 2 == 0:
                                nc.vector.tensor_scalar_mul(
                                    otp[:, 0, dc * NC:(dc + 1) * NC], ps[:],
                                    inv[:, it:it + 1])
                            else:
                                nc.scalar.mul(otp[:, 0, dc * NC:(dc + 1) * NC], ps[:],
                                              inv[:, it:it + 1])
                        gi = ic * JC + it
                        q = nc.sync if warm else nc.gpsimd
                        q.dma_start(out_r[:, gi:gi + 1, :], otp[:])

                def emit_attention(ic):
                    isl = slice(ic * NC, (ic + 1) * NC)
                    av_ps = [psavpool.tile([P, NC], F32, tag="av", name=f"av_{rt}")
                             for rt in range(RT)]
                    acc = accpool.tile([P, NC], F32, tag="acc", name="acc")
                    ets = {}

                    def step(jt):
                        do_at, do_ev = jt < JT, jt >= LAG
                        ej = jt - LAG
                        if do_at:
                            ps = ps_tile("ps_at")
                        if do_ev:
                            et = ets.pop(ej)
                        if do_at:
                            nc.tensor.matmul(ps[:], KT_t[:, 0, jt * P:(jt + 1) * P],
                                             QT_t[:, 0, isl], start=True, stop=False)
                        if do_ev:
                            nc.tensor.matmul(av_ps[0][:], V_t[:, ej, 0:P], et[:],
                                             start=(ej == 0), stop=(ej == JT - 1))
                        if do_at:
                            nc.tensor.matmul(ps[:], KT_t[:, 1, jt * P:(jt + 1) * P],
                                             QT_t[:, 1, isl], start=False, stop=True)
                        if do_ev:
                            nc.tensor.matmul(av_ps[1][:], V_t[:, ej, P:RANK], et[:],
                                             start=(ej == 0), stop=(ej == JT - 1))
                        if do_at:
                            etn = etpool.tile([P, NC], BF16, tag="et", name="et")
                            nc.scalar.activation(etn[:], ps[:], AF.Exp, scale=float(SCALE))
                            ets[jt] = etn
                        if do_ev:
                            if ej == 0:
                                nc.vector.tensor_copy(acc[:], et[:])
                            else:
                                nc.vector.tensor_tensor(acc[:], acc[:], et[:], ADD)

                    for jt in range(JT + LAG):
                        step(jt)
                    pending[ic] = (av_ps, acc)

                for ic in range(SC):
                    emit_qproj(ic)
                    if ic > 0:
                        emit_outproj(ic - 1)
                    emit_attention(ic)
                emit_outproj_last(SC - 1)

            if reps == 1:
                body()
            else:
                # Steady-state loop: PE stays warm across iterations, so the
                # p-state warmup matmuls would be pure per-iteration overhead.
                with tc.For_i(0, reps, 1) as i:
                    body(i, warm=True)

    nc.compile()
    return nc


_CACHE = {}


def _get_program():
    if "nc" not in _CACHE:
        _CACHE["nc"] = build_program(reps=1)
    return _CACHE["nc"]


def _bf16(x):
    return np.asarray(x, dtype=np.float32).astype(ml_dtypes.bfloat16)


def prep_in_maps(q, k, v, Wq, Wk, Wv, Wo):
    # Zero-FLOP host-side layout/dtype prep: transpose so the contraction dim
    # (D) lands on SBUF partitions, cast to bf16; one batch per core.
    qT = np.asarray(q, dtype=np.float32).transpose(0, 2, 1).astype(ml_dtypes.bfloat16)
    kT = np.asarray(k, dtype=np.float32).transpose(0, 2, 1).astype(ml_dtypes.bfloat16)
    vT = np.asarray(v, dtype=np.float32).transpose(0, 2, 1).astype(ml_dtypes.bfloat16)
    wqT = _bf16(Wq).T.copy()
    wkT = _bf16(Wk).T.copy()
    wvT = _bf16(Wv).T.copy()
    woT = _bf16(Wo).T.copy()
    return [{"qT": qT[c], "kT": kT[c], "vT": vT[c],
             "wqT": wqT, "wkT": wkT, "wvT": wvT, "woT": woT}
            for c in range(B)]


def kernel(q, k, v, Wq, Wk, Wv, Wo):
    nc = _get_program()
    in_maps = prep_in_maps(q, k, v, Wq, Wk, Wv, Wo)
    res = bass_utils.run_bass_kernel_spmd(nc, in_maps, core_ids=list(range(B)))
    return np.stack([res.results[c]["out"] for c in range(B)], axis=0).astype(np.float32)
